# revision 25
# baseline (speedup 1.0000x reference)
"""TRN2 Bass kernel for nn_Attention_20633022890922.

The reference module's einsum 'bqhk,bvhd->bqhd' contracts the attention-weight
head axis (k) and the value head axis (v) independently, so the product
factorizes into (sum_k softmax(...)) * (sum_v V) = 1 * Vsum.  The whole module
is therefore algebraically a single linear layer:

    out = tokens @ Wv_sum @ Wo_sum + bo
      Wv_sum[h, d]  = sum_v Wv[h, v*64 + d]          (512 x 64)
      Wo_sum[d, e]  = sum_q Wo[q*64 + d, e]          (64 x 512)

(The only approximation is softmax summing to 1.0, which holds to ~1e-7 in
fp32.)  Wq / Wk cancel entirely.

Device strategy: data-parallel over the batch dim (8 batches -> 8 cores).
Per core: Y = X @ Wv_sum @ Wo_sum with X [8192, 512]; bo added on host.

The profile of the previous version showed the kernel hard DMA-bound (SDMA
active 99.3% of the span, 25.6 MB of HBM traffic at ~290 GB/s against a
~358 GB/s per-core HBM limit), so this version minimizes bytes moved:

  - X is cast to fp16 AND pre-transposed on the host into per-chunk
    hid-major layout [chunk, 128 hid, 4 blk, 512 tok], so every input DMA
    is one dense contiguous 512 KiB transfer (8 MiB total).
  - Y is written as fp16 (8 MiB instead of 16 MiB fp32) in the PSUM-native
    [chunk, 128 tok-partition, 4 tile, 512 hid] layout, one dense 512 KiB
    store per chunk; the host un-permutes and upcasts.  fp16 quantization
    of Y costs ~2.4e-4 max-rel, far inside the 2e-2 gate.
  - Weights are folded on the host and shipped fp16 (PE quantizes matmul
    operands to ~12 mantissa bits anyway; wv/wo fp16 rounding is ~1e-4).

  GEMM1 (per 512-token chunk): pt = Wv_dup.T @ X^T, 4 accumulating
        matmuls (one PSUM bank caps the output at 512 fp32 columns);
        stationary 128-col wv keeps FWL, stationary-outer over 2-chunk
        groups.
  GEMM2 (per 128-token tile, K=64): Y[t, :] = T @ Wo_sum, stationary
        tt[0:64, tile] (the unique T rows), streaming wop [64, 512].

  The PE issue order is software-pipelined (G1(0), G1(1), G2(0), G1(2),
  G2(1), ...) so the tensor engine streams matmuls back-to-back and stays
  HAM-warm instead of micro-idling at K=4/8 half clock, while Vector and
  Scalar cast the previous super-chunk's T out of PSUM.  A short burst of
  scratch matmuls at kernel start warms the HAM clock gate to 8/8 during
  the otherwise-dead DMA ramp (~7 us of framework preamble + first loads)
  so the first real matmuls run at full clock.  The PSUM->SBUF fp16 casts
  (GpSimd has no PSUM port) are split ~50/50 across Vector and Scalar,
  and the per-chunk stores go out via GpSimd/SWDGE so no engine doubles
  as both caster and DMA trigger.
"""

import time

import numpy as np

from concourse import bacc, mybir, tile
from concourse import bass_utils

B, N_TOK, HID, EMB, NH, HD = 8, 8192, 512, 512, 8, 64
N_CORES = 8
CH = 512                      # tokens per store chunk
NCHUNK = N_TOK // CH          # 16
SC = 1024                     # tokens per super-chunk (matmul moving width)
NSC = N_TOK // SC             # 8
WARM_MM = 6                   # scratch matmuls to pre-warm the PE clock

F32 = mybir.dt.float32
FP16 = mybir.dt.float16

_compiled = None


def _build():
    nc = bacc.Bacc(
        trn_type="TRN2", target_bir_lowering=False, debug=False, num_devices=N_CORES
    )

    # host-transposed fp16 X, blocked: [8, 128 hid-in-blk, 4 blk, 1024 tok]
    xf_d = nc.dram_tensor("xf", [NSC, 128, 4, SC], FP16, kind="ExternalInput")
    # packed consts: cols 0:512 wv_dup (4 blocks x 128 cols), cols 512:1024
    # wop (rows 0:64 = Wo_sum fp16, rest zero)
    cw_d = nc.dram_tensor("cw", [128, 1024], FP16, kind="ExternalInput")
    # fp16 Y in PSUM-native layout: [16, 128 tok-par, 4 tok-tile, 512 hid]
    y_d = nc.dram_tensor("y", [NCHUNK, 128, 4, HID], FP16, kind="ExternalOutput")

    with tile.TileContext(nc) as tc:
        with (
            tc.tile_pool(name="const", bufs=1) as constp,
            tc.tile_pool(name="warm", bufs=1) as warm_p,
            tc.tile_pool(name="xt", bufs=NSC) as xt_p,
            tc.tile_pool(name="tt", bufs=4) as tt_p,
            tc.tile_pool(name="yout", bufs=8) as y_p,
            tc.tile_pool(name="ps_t", bufs=4, space="PSUM") as ps_t,
            tc.tile_pool(name="ps_y", bufs=4, space="PSUM") as ps_y,
        ):
            cw = constp.tile([128, 1024], FP16, tag="cw")
            # split const load: the first GEMM1 matmuls only need wv
            nc.scalar.dma_start(cw[:, 0:512], cw_d[:, 0:512])
            nc.scalar.dma_start(cw[:, 512:1024], cw_d[:, 512:1024])
            wop = cw[0:64, 512:1024]

            # ---- all input loads issued up front.  The first two super-
            # chunks are split per 512-token chunk so GEMM1 can start
            # ~1.5 us earlier; the rest are dense 1 MiB transfers.
            xts = []
            for s in range(NSC):
                t = xt_p.tile([128, 4, SC], FP16, tag="xt", name=f"xt{s}")
                if s < 2:
                    nc.sync.dma_start(t[:, :, 0:CH], xf_d[s][:, :, 0:CH])
                    nc.sync.dma_start(t[:, :, CH:SC], xf_d[s][:, :, CH:SC])
                else:
                    nc.sync.dma_start(t[:], xf_d[s])
                xts.append(t)

            # ---- PE clock warmup: the HAM gate starts the PE at half
            # clock and needs ~4us of sustained activity to open to 8/8.
            # Run scratch matmuls while the first loads are in flight so
            # the real matmuls start at full clock.  Scratch SBUF is
            # memset by Vector (idle then); the PSUM tile is a dead tile
            # from the pt pool, so Tile orders pt reuse after the warmup.
            wsrc = warm_p.tile([128, 640], FP16, tag="wsrc")
            nc.vector.memset(wsrc[:], 0.0)
            pwarm = ps_t.tile([128, CH], F32, tag="pt", name="pt_warm")
            for w in range(WARM_MM):
                nc.tensor.matmul(
                    pwarm[:], wsrc[:, 0:128], wsrc[:, 128:640],
                    start=True, stop=True, skip_group_check=True,
                )

            def cast_tt(c, pt):
                # PSUM->SBUF fp16 cast of the unique T rows; engine cost
                # scales with free size, so alternate engines per chunk
                tt = tt_p.tile([64, CH], FP16, tag="tt", name=f"tt{c}")
                if c % 2 == 0:
                    nc.vector.tensor_copy(tt[:], pt[0:64, :])
                else:
                    nc.scalar.copy(tt[:], pt[0:64, :])
                return tt

            def gemm1(c, pt, j):
                s, off = c // 2, (c % 2) * CH
                nc.tensor.matmul(
                    pt[:], cw[:, j * 128:(j + 1) * 128],
                    xts[s][:, j, off:off + CH],
                    start=(j == 0), stop=(j == 3), skip_group_check=True,
                )

            def gemm2_and_store(c, tt):
                # stationary = tt token-tile, moving = wop [64, 512];
                # py = [128 tok, 512 hid]
                if c < NCHUNK - 1:
                    yo = y_p.tile([128, 4, HID], FP16, tag="yo",
                                  name=f"yo{c}")
                    for i in range(4):
                        py = ps_y.tile([128, HID], F32, tag="py")
                        nc.tensor.matmul(
                            py[:], tt[:, 128 * i:128 * (i + 1)], wop,
                            start=True, stop=True,
                        )
                        # 2 casts on Vector, 2 on Scalar per chunk
                        if i % 2 == (c % 2):
                            nc.vector.tensor_copy(yo[:, i, :], py[:])
                        else:
                            nc.scalar.copy(yo[:, i, :], py[:])
                    # loads are long done by the second half: use the
                    # lower-latency HWDGE (sync) ring there, SWDGE before
                    eng = nc.gpsimd if c < 12 else nc.sync
                    eng.dma_start(y_d[c], yo[:])
                else:
                    # final chunk: 4 independent mini-pipelines
                    # (MM -> cast -> store) on alternating engines/rings
                    # so the last-byte receipt chain is as short as
                    # possible
                    for i in range(4):
                        py = ps_y.tile([128, HID], F32, tag="py")
                        nc.tensor.matmul(
                            py[:], tt[:, 128 * i:128 * (i + 1)], wop,
                            start=True, stop=True,
                        )
                        ym = y_p.tile([128, HID], FP16, tag="ym",
                                      name=f"ym{i}")
                        if i % 2 == 0:
                            nc.vector.tensor_copy(ym[:], py[:])
                            nc.sync.dma_start(y_d[c, :, i, :], ym[:])
                        else:
                            nc.scalar.copy(ym[:], py[:])
                            nc.scalar.dma_start(y_d[c, :, i, :], ym[:])

            # Software pipeline over 2-chunk groups: PE issue order
            # G1(0), G1(1), G2(0), G1(2), G2(1), ... so Vector/Scalar
            # cast T of group g out of PSUM while the PE streams G1(g+1),
            # and the PE never waits on a cast.  PSUM: pt 4 + py 4 = 8.
            tts = {}
            for g in range(NSC + 1):
                if g < NSC:
                    chunks = range(2 * g, 2 * g + 2)
                    pts = {c: ps_t.tile([128, CH], F32, tag="pt",
                                        name=f"pt{c}") for c in chunks}
                    # weight-stationary-outer: 4 switches per group
                    for j in range(4):
                        for c in chunks:
                            gemm1(c, pts[c], j)
                    for c in chunks:
                        tts[c] = cast_tt(c, pts.pop(c))
                if g > 0:
                    for c in range(2 * (g - 1), 2 * g):
                        gemm2_and_store(c, tts.pop(c))

    nc.compile()
    return nc


def _get_compiled():
    global _compiled
    if _compiled is None:
        _compiled = _build()
    return _compiled


def kernel(tokens, Wq, Wk, Wv, Wo, bo, _trace=False):
    tokens = np.asarray(tokens, dtype=np.float32)
    Wv = np.asarray(Wv, dtype=np.float32)
    Wo = np.asarray(Wo, dtype=np.float32)
    bo = np.asarray(bo, dtype=np.float32)

    # Host-side prep: fold weights, cast X to fp16 and pre-transpose into
    # the chunk-blocked hid-major device layout (all device DMAs dense).
    wv_sum = Wv.reshape(HID, NH, HD).sum(axis=1)               # [512, 64]
    wo_sum = Wo.reshape(NH, HD, HID).sum(axis=0)               # [64, 512]
    # duplicate wv output cols -> 128-col stationary (keeps FWL), then to
    # on-chip [k-within-block, blk*128 + m] layout
    wvd = np.concatenate([wv_sum, wv_sum], axis=1)             # [512, 128]
    wv_chip = wvd.reshape(4, 128, 128).transpose(1, 0, 2).reshape(128, 512)
    wop = np.zeros((128, 512), dtype=np.float32)
    wop[0:64] = wo_sum
    cw = np.ascontiguousarray(
        np.concatenate([wv_chip, wop], axis=1).astype(np.float16)
    )                                                          # [128, 1024]

    xf = tokens.astype(np.float16)                             # [B, 8192, 512]
    # -> [B, super-chunk, 128 hid-in-blk, 4 blk, 1024 tok]
    xf = np.ascontiguousarray(
        xf.reshape(B, NSC, SC, 4, 128).transpose(0, 1, 4, 3, 2)
    )

    nc = _get_compiled()
    in_maps = [{"xf": xf[b], "cw": cw} for b in range(N_CORES)]
    # retry once or twice on transient device flakes (rare NRT_EXEC_UNIT
    # wedges have been observed under the axon PJRT path)
    for attempt in range(3):
        try:
            res = bass_utils.run_bass_kernel_spmd(
                nc, in_maps, core_ids=list(range(N_CORES)), trace=_trace
            )
            break
        except Exception:
            if attempt == 2:
                raise
            time.sleep(20)
    ys = np.stack([res.results[b]["y"] for b in range(N_CORES)], axis=0)
    # y_d[c, p, i, h] = y[c*512 + i*128 + p, h] (token-major GEMM2 out):
    # [B, 16, 128, 4, 512] -> [B, 16, 4, 128, 512] -> [B, 8192, 512]
    out = np.ascontiguousarray(ys.transpose(0, 1, 3, 2, 4)).reshape(
        B, N_TOK, HID
    ).astype(np.float32)
    if np.any(bo):
        out += bo
    if _trace:
        return out, res
    return out


if __name__ == "__main__":
    rng = np.random.default_rng(0)
    ins = {
        "tokens": rng.standard_normal((B, N_TOK, HID)).astype(np.float32),
        "Wq": (rng.standard_normal((HID, EMB)) * 0.02).astype(np.float32),
        "Wk": (rng.standard_normal((HID, EMB)) * 0.02).astype(np.float32),
        "Wv": (rng.standard_normal((HID, HID)) * 0.02).astype(np.float32),
        "Wo": (rng.standard_normal((EMB, HID)) * 0.02).astype(np.float32),
        "bo": np.zeros((HID,), dtype=np.float32),
    }
    out = kernel(**ins)
    print(out.shape, out.dtype)
